# revision 2
# baseline (speedup 1.0000x reference)
"""PointSIFT (select_cube + 4x conv_bn) Trainium2 Bass kernel.

Strategy: data-parallel over (batch, quarter-of-N): 8 cores, core g owns
batch g//4 and sorted-query range [1024*(g%4), +1024).

Host: sort points by x per batch; for each 128-query chunk build a 1280-wide
candidate window (all points within x-radius of the chunk are inside it).
Device per chunk: replicate window coords (K=1 matmul, exact), subtract query
coords (ACT bias-add, exact), then a custom-DVE pipeline computes the masked
per-octant min cube distance bit-exactly in the reference's fp32 op order;
max_index recovers argmins; selector matmuls re-layout the indices; ap_gather
gathers neighbor features; PE runs the 4-layer conv/BN/ReLU MLP with BN scale
folded into weights and (b-mean)*s+beta applied as ACT per-partition bias.
"""
import sys

sys.path.insert(0, "/opt/trn_rl_repo")
import numpy as np

B = 2
N = 4096
W = 1280
R = np.float32(0.1)
EPS = np.float32(1e-5)
NCORES = 8

_TRACE = False
_LAST_EXEC_NS = None
_PROG = None


# ---------------------------------------------------------------- DVE ops
def _register_dve_ops():
    from concourse import dve_ops
    from concourse.dve_spec import (
        C0,
        C1,
        C2,
        AluOp,
        Bin,
        Spec,
        Src0,
        Src1,
        Zero,
        _has_src1,
        lower,
        minn,
        select,
    )
    from concourse.dve_uop import DveOpSpec

    if hasattr(dve_ops, "PSIFT_P1"):
        return

    f32 = np.float32

    def _ref_p1(in0, in1, s0, s1, imm2):
        m = (in0 * in0 + in1 * in1).astype(f32)
        fl = (((s0 >= in0).astype(f32) + (s0 >= in1).astype(f32)) * s1).astype(f32)
        return (m + fl).astype(f32)

    def _ref_p2(in0, in1, s0, s1, imm2):
        a = (in1 + in0 * in0).astype(f32)
        fl = (((in0 >= s0).astype(f32) + (s1 >= in0).astype(f32)) * imm2).astype(f32)
        return (a + fl).astype(f32)

    def _ref_p3(in0, in1, s0, s1, imm2):
        t = ((in0 > 0).astype(f32) + (in1 > 0).astype(f32) * s0).astype(f32)
        return (t + (in0 >= s1).astype(f32) * imm2).astype(f32)

    def _ref_p4(in0, in1, s0, s1, imm2):
        return ((in0 + in0) + (in1 > 0).astype(f32)).astype(f32)

    def _ref_axpyge(in0, in1, s0, s1, imm2):
        return (in0 + (in1 >= s1).astype(f32) * s0).astype(f32)

    def _ref_octsel(in0, in1, s0, s1, imm2):
        body = np.where(in1 == s0, in0, s1).astype(f32)
        acc = np.minimum(
            body.reshape(body.shape[0], -1).min(axis=-1, keepdims=True), s1
        ).astype(f32)
        return body, acc

    defs = [
        # sp = (dx*dx + dy*dy) + ((s0>=dx) + (s0>=dy))*s1
        ("PSIFT_P1",
         Spec(body=(Src0 * Src0 + Src1 * Src1)
              + (Bin(AluOp.IS_GE, C0, Src0) + Bin(AluOp.IS_GE, C0, Src1)) * C1,
              reference=_ref_p1)),
        # dp = (sp + dz*dz) + ((dz>=s0) + (s1>=dz))*imm2
        ("PSIFT_P2",
         Spec(body=(Src1 + Src0 * Src0)
              + (Bin(AluOp.IS_GE, Src0, C0) + Bin(AluOp.IS_GE, C1, Src0)) * C2,
              reference=_ref_p2)),
        # t = ((dx>0) + (dy>0)*s0) + (dx>=s1)*imm2
        ("PSIFT_P3",
         Spec(body=(Bin(AluOp.IS_GT, Src0, Zero) + Bin(AluOp.IS_GT, Src1, Zero) * C0)
              + Bin(AluOp.IS_GE, Src0, C1) * C2,
              reference=_ref_p3)),
        # e = (t + t) + (dz>0)
        ("PSIFT_P4",
         Spec(body=(Src0 + Src0) + Bin(AluOp.IS_GT, Src1, Zero),
              reference=_ref_p4)),
        # y = x + (m>=s1)*s0
        ("PSIFT_AXPYGE",
         Spec(body=Src0 + Bin(AluOp.IS_GE, Src1, C1) * C0,
              reference=_ref_axpyge)),
        # v = min over row of (oct==s0 ? dist : s1), min'd with s1
        ("PSIFT_OCTSEL",
         Spec(body=select(Bin(AluOp.IS_EQ, Src1, C0), Src0, C1),
              accum=minn, accum_init=C1,
              reference=_ref_octsel)),
    ]

    for name, spec in defs:
        row = max(dve_ops._SUB_OPCODE_FOR_NAME.values()) + 1
        assert row < 0x20
        dve_ops._SUB_OPCODE_FOR_NAME[name] = row
        shas = {}
        for ver in ("v3", "v4"):
            try:
                shas[ver] = DveOpSpec(
                    name=name, opcode=row, uops=lower(spec, ver=ver),
                    rd1_en=_has_src1(spec),
                ).sha(ver)
            except Exception:
                pass
        op = dve_ops.DveOp(name=name, spec=spec, subdim=False, uops_sha=shas)
        dve_ops.OPS.append(op)
        dve_ops.CUSTOM_DVE_SPECS[name] = spec
        setattr(dve_ops, name, op)


# ---------------------------------------------------------------- program
def _build_program():
    from concourse import bacc, mybir, tile
    from concourse import dve_ops

    _register_dve_ops()
    P1 = dve_ops.PSIFT_P1
    P2 = dve_ops.PSIFT_P2
    P3 = dve_ops.PSIFT_P3
    P4 = dve_ops.PSIFT_P4
    AXPYGE = dve_ops.PSIFT_AXPYGE
    OCTSEL = dve_ops.PSIFT_OCTSEL

    f32 = mybir.dt.float32
    Alu = mybir.AluOpType
    Act = mybir.ActivationFunctionType

    nc = bacc.Bacc("TRN2", target_bir_lowering=False, debug=True)

    feat_d = nc.declare_dram_parameter("feat", [80, N], f32, isOutput=False)
    rhs_d = nc.declare_dram_parameter("rhs", [8, 3 * W], f32, isOutput=False)
    qneg_d = nc.declare_dram_parameter("qneg", [128, 24], f32, isOutput=False)
    lo_d = nc.declare_dram_parameter("lo", [128, 8], f32, isOutput=False)
    spq_d = nc.declare_dram_parameter("spq", [128, 64], f32, isOutput=False)
    xyzq_d = nc.declare_dram_parameter("xyzq", [3, 1024], f32, isOutput=False)
    eye_d = nc.declare_dram_parameter("eye", [128, 128], f32, isOutput=False)
    rep_d = nc.declare_dram_parameter("rep16", [16, 128], f32, isOutput=False)
    w0a_d = nc.declare_dram_parameter("w0a", [67, 128], f32, isOutput=False)
    w0b_d = nc.declare_dram_parameter("w0b", [67, 128], f32, isOutput=False)
    wc3_d = nc.declare_dram_parameter("wc3", [3, 128], f32, isOutput=False)
    w1a_d = nc.declare_dram_parameter("w1a", [128, 128], f32, isOutput=False)
    w1b_d = nc.declare_dram_parameter("w1b", [128, 128], f32, isOutput=False)
    w2a_d = nc.declare_dram_parameter("w2a", [128, 128], f32, isOutput=False)
    w2b_d = nc.declare_dram_parameter("w2b", [128, 128], f32, isOutput=False)
    w3_d = nc.declare_dram_parameter("w3", [128, 128], f32, isOutput=False)
    bi_d = nc.declare_dram_parameter("bi", [128, 4], f32, isOutput=False)
    out_d = nc.declare_dram_parameter("out", [128, 1024], f32, isOutput=True)
    pos_d = nc.declare_dram_parameter("pos", [128, 64], f32, isOutput=True)

    dma = nc.default_dma_engine

    with tile.TileContext(nc) as tc, tc.tile_pool(name="sb", bufs=1) as sb:
        feat_s = sb.tile([80, N], f32)
        dma.dma_start(out=feat_s[:], in_=feat_d[:])
        qneg_s = sb.tile([128, 24], f32)
        dma.dma_start(out=qneg_s[:], in_=qneg_d[:])
        lo_s = sb.tile([128, 8], f32)
        dma.dma_start(out=lo_s[:], in_=lo_d[:])
        spq_s = sb.tile([128, 64], f32)
        dma.dma_start(out=spq_s[:], in_=spq_d[:])
        xyzq_s = sb.tile([3, 1024], f32)
        dma.dma_start(out=xyzq_s[:], in_=xyzq_d[:])
        eye_s = sb.tile([128, 128], f32)
        dma.dma_start(out=eye_s[:], in_=eye_d[:])
        rep_s = sb.tile([16, 128], f32)
        dma.dma_start(out=rep_s[:], in_=rep_d[:])
        w0a_s = sb.tile([67, 128], f32)
        dma.dma_start(out=w0a_s[:], in_=w0a_d[:])
        w0b_s = sb.tile([67, 128], f32)
        dma.dma_start(out=w0b_s[:], in_=w0b_d[:])
        wc3_s = sb.tile([3, 128], f32)
        dma.dma_start(out=wc3_s[:], in_=wc3_d[:])
        w1a_s = sb.tile([128, 128], f32)
        dma.dma_start(out=w1a_s[:], in_=w1a_d[:])
        w1b_s = sb.tile([128, 128], f32)
        dma.dma_start(out=w1b_s[:], in_=w1b_d[:])
        w2a_s = sb.tile([128, 128], f32)
        dma.dma_start(out=w2a_s[:], in_=w2a_d[:])
        w2b_s = sb.tile([128, 128], f32)
        dma.dma_start(out=w2b_s[:], in_=w2b_d[:])
        w3_s = sb.tile([128, 128], f32)
        dma.dma_start(out=w3_s[:], in_=w3_d[:])
        bi_s = sb.tile([128, 4], f32)
        dma.dma_start(out=bi_s[:], in_=bi_d[:])

        ones1 = sb.tile([1, 128], f32)
        nc.vector.memset(ones1[:], 1.0)

        sp_t = sb.tile([128, W], f32)
        dp_t = sb.tile([128, W], f32)
        tt_t = sb.tile([128, W], f32)
        u_t = sb.tile([128, W], f32)
        oc_t = sb.tile([128, W], f32)
        dd_t = sb.tile([128, W], f32)
        sel_t = sb.tile([128, W], f32)
        v8_s = sb.tile([128, 8], f32)
        idx8_s = sb.tile([128, 8], mybir.dt.uint16)
        widxf_s = sb.tile([128, 8], f32)
        posf_s = sb.tile([128, 8], f32)
        emp_s = sb.tile([128, 8], mybir.dt.uint8)
        pos_all = sb.tile([128, 64], f32)

        # ---------------- Phase A: per-chunk neighbor search
        with tc.tile_pool(name="lp", bufs=2) as lp, \
             tc.tile_pool(name="psA", bufs=2, space="PSUM") as psA:
            for c in range(8):
                win1 = lp.tile([1, 3 * W], f32, name="win1")
                dma.dma_start(out=win1[:], in_=rhs_d[c:c + 1, :])
                dxyz = lp.tile([128, 3 * W], f32, name="dxyz")
                for k in range(3):
                    for b0, bw in ((0, 512), (512, 512), (1024, 256)):
                        pt = psA.tile([128, 512], f32, name="pt")
                        nc.tensor.matmul(
                            pt[:, 0:bw], ones1[:],
                            win1[0:1, W * k + b0: W * k + b0 + bw],
                            start=True, stop=True)
                        nc.scalar.activation(
                            dxyz[:, W * k + b0: W * k + b0 + bw], pt[:, 0:bw],
                            Act.Identity,
                            bias=qneg_s[:, 3 * c + k: 3 * c + k + 1], scale=1.0)

                dx = dxyz[:, 0:W]
                dy = dxyz[:, W:2 * W]
                dz = dxyz[:, 2 * W:3 * W]
                cd = nc.vector._custom_dve
                cd(P1, out=sp_t[:], in0=dx, in1=dy, s0=-0.1, s1=100.0)
                cd(P2, out=dp_t[:], in0=dz, in1=sp_t[:], s0=0.1, s1=-0.1,
                   imm2=100.0)
                cd(P3, out=tt_t[:], in0=dx, in1=dy, s0=2.0, s1=0.1, imm2=8.0)
                cd(P4, out=u_t[:], in0=tt_t[:], in1=dz)
                cd(AXPYGE, out=oc_t[:], in0=u_t[:], in1=dy, s0=8.0, s1=0.1)
                cd(AXPYGE, out=dd_t[:], in0=dp_t[:], in1=oc_t[:], s0=100.0,
                   s1=8.0)
                for o in range(8):
                    bx, by, bz = (o >> 2) & 1, (o >> 1) & 1, o & 1
                    e_o = float(2 * bx + 4 * by + bz)
                    cd(OCTSEL, out=sel_t[:], in0=dd_t[:], in1=oc_t[:],
                       s0=e_o, s1=1e9, accum_out=v8_s[:, o:o + 1])
                nc.vector.max_index(idx8_s[:], v8_s[:], dd_t[:])
                nc.vector.tensor_copy(widxf_s[:], idx8_s[:])
                nc.vector.tensor_scalar(emp_s[:], v8_s[:], 1.0, None,
                                        op0=Alu.is_ge)
                nc.vector.tensor_scalar(posf_s[:], widxf_s[:],
                                        lo_s[:, c:c + 1], None, op0=Alu.add)
                nc.vector.select(pos_all[:, 8 * c:8 * c + 8], emp_s[:],
                                 spq_s[:, 8 * c:8 * c + 8], posf_s[:])

        dma.dma_start(out=pos_d[:], in_=pos_all[:])

        # ---------------- Phase B: index re-layout + gather
        idxg_s = sb.tile([16, 512], f32)
        idxi_s = sb.tile([128, 512], mybir.dt.int16)
        g_s = sb.tile([80, 8192], f32)
        with tc.tile_pool(name="psB", bufs=1, space="PSUM") as psB:
            pperm = psB.tile([16, 512], f32, name="pperm")
            for j in range(8):
                nc.tensor.matmul(pperm[:, 64 * j:64 * j + 64],
                                 eye_s[:, 16 * j:16 * j + 16], pos_all[:],
                                 start=True, stop=True)
            for o in range(8):
                src = pperm[0:16, o:o + 505:8].rearrange(
                    "p (j c) -> p c j", j=8, c=8)
                dst = idxg_s[:, 64 * o:64 * o + 64].rearrange(
                    "p (c j) -> p c j", c=8, j=8)
                nc.vector.tensor_copy(dst, src)
            prep2 = psB.tile([128, 512], f32, name="prep2")
            nc.tensor.matmul(prep2[:], rep_s[:], idxg_s[:], start=True,
                             stop=True)
            nc.vector.tensor_copy(idxi_s[:], prep2[:])
        nc.gpsimd.ap_gather(g_s[:], feat_s[:], idxi_s[0:80, :], channels=80,
                            num_elems=N, d=1, num_idxs=8192)

        # ---------------- Phase C: conv/BN/ReLU MLP
        y1_s = sb.tile([128, 2048], f32)
        y2_s = sb.tile([128, 1024], f32)
        y3_s = sb.tile([128, 512], f32)
        outp_s = sb.tile([128, 1024], f32)
        with tc.tile_pool(name="psC", bufs=1, space="PSUM") as psC:
            for qb in range(2):
                q0 = 512 * qb
                for t in range(4):
                    p0 = psC.tile([128, 512], f32, name="p0")
                    nc.tensor.matmul(p0[:], w0a_s[:],
                                     g_s[0:67, 2048 * t + q0:2048 * t + q0 + 512],
                                     start=True, stop=False)
                    nc.tensor.matmul(p0[:], w0b_s[:],
                                     g_s[0:67, 2048 * t + 1024 + q0:
                                         2048 * t + 1024 + q0 + 512],
                                     start=False, stop=False)
                    nc.tensor.matmul(p0[:], wc3_s[:], xyzq_s[:, q0:q0 + 512],
                                     start=False, stop=True)
                    nc.scalar.activation(y1_s[:, 512 * t:512 * t + 512], p0[:],
                                         Act.Relu, bias=bi_s[:, 0:1], scale=1.0)
                for u in range(2):
                    p1 = psC.tile([128, 512], f32, name="p1")
                    nc.tensor.matmul(p1[:], w1a_s[:],
                                     y1_s[:, 1024 * u:1024 * u + 512],
                                     start=True, stop=False)
                    nc.tensor.matmul(p1[:], w1b_s[:],
                                     y1_s[:, 1024 * u + 512:1024 * u + 1024],
                                     start=False, stop=True)
                    nc.scalar.activation(y2_s[:, 512 * u:512 * u + 512], p1[:],
                                         Act.Relu, bias=bi_s[:, 1:2], scale=1.0)
                p2 = psC.tile([128, 512], f32, name="p2")
                nc.tensor.matmul(p2[:], w2a_s[:], y2_s[:, 0:512], start=True,
                                 stop=False)
                nc.tensor.matmul(p2[:], w2b_s[:], y2_s[:, 512:1024],
                                 start=False, stop=True)
                nc.scalar.activation(y3_s[:], p2[:], Act.Relu,
                                     bias=bi_s[:, 2:3], scale=1.0)
                p3 = psC.tile([128, 512], f32, name="p3")
                nc.tensor.matmul(p3[:], w3_s[:], y3_s[:], start=True, stop=True)
                nc.scalar.activation(outp_s[:, q0:q0 + 512], p3[:], Act.Relu,
                                     bias=bi_s[:, 3:4], scale=1.0)
        dma.dma_start(out=out_d[:], in_=outp_s[:])

    nc.finalize()
    return nc


# ---------------------------------------------------------------- host
def _prep_weights(params):
    f32 = np.float32
    ws = {}
    scales = []
    for li, key in enumerate(("c1_0", "c1_1", "c1_2", "c2")):
        p = params[key]
        Wt = np.asarray(p["W"], f32)
        s = (np.asarray(p["gamma"], f32)
             / np.sqrt(np.asarray(p["var"], f32) + EPS)).astype(f32)
        scales.append(s)
        ws[key] = Wt * s[:, None, None, None]
    bi = np.zeros((128, 4), f32)
    for li, key in enumerate(("c1_0", "c1_1", "c1_2", "c2")):
        p = params[key]
        bi[:, li] = ((np.asarray(p["b"], f32) - np.asarray(p["mean"], f32))
                     * scales[li] + np.asarray(p["beta"], f32))
    w0 = ws["c1_0"]
    out = {
        "w0a": np.ascontiguousarray(w0[:, :, 0, 0].T),
        "w0b": np.ascontiguousarray(w0[:, :, 0, 1].T),
        "wc3": np.ascontiguousarray(-(w0[:, 0:3, 0, 0] + w0[:, 0:3, 0, 1]).T),
        "w1a": np.ascontiguousarray(ws["c1_1"][:, :, 0, 0].T),
        "w1b": np.ascontiguousarray(ws["c1_1"][:, :, 0, 1].T),
        "w2a": np.ascontiguousarray(ws["c1_2"][:, :, 0, 0].T),
        "w2b": np.ascontiguousarray(ws["c1_2"][:, :, 0, 1].T),
        "w3": np.ascontiguousarray(ws["c2"][:, :, 0, 0].T),
        "bi": bi,
    }
    return out


def kernel(xyz, points, params):
    global _PROG, _LAST_EXEC_NS
    from concourse.bass_utils import run_bass_kernel_spmd

    f32 = np.float32
    xyz = np.asarray(xyz, f32)
    points = np.asarray(points, f32)
    if _PROG is None:
        _PROG = _build_program()
    nc = _PROG

    wmaps = _prep_weights(params)
    eye = np.eye(128, dtype=f32)
    rep16 = np.tile(np.eye(16, dtype=f32), (1, 8))

    in_maps = []
    orders = []
    for b in range(B):
        order = np.argsort(xyz[b, :, 0], kind="stable")
        orders.append(order)
        xs = xyz[b][order]
        xsp = np.concatenate([xs, np.full((W, 3), 1000.0, f32)], axis=0)
        feat = np.zeros((80, N), f32)
        feat[0:3] = xs.T
        feat[3:67] = points[b][order].T
        for quarter in range(4):
            qstart = 1024 * quarter
            rhs = np.empty((8, 3 * W), f32)
            lo_arr = np.empty(8, f32)
            for c in range(8):
                q0 = qstart + 128 * c
                lo = int(np.searchsorted(
                    xs[:, 0], f32(xs[q0, 0] - R), side="left"))
                lo_arr[c] = lo
                win = xsp[lo:lo + W]
                rhs[c, 0:W] = win[:, 0]
                rhs[c, W:2 * W] = win[:, 1]
                rhs[c, 2 * W:3 * W] = win[:, 2]
            qs = xs[qstart:qstart + 1024].reshape(8, 128, 3)
            qneg = np.ascontiguousarray(
                (-qs.transpose(1, 0, 2)).reshape(128, 24))
            posq = np.arange(qstart, qstart + 1024, dtype=f32).reshape(8, 128)
            spq = np.ascontiguousarray(
                np.repeat(posq.T[:, :, None], 8, axis=2).reshape(128, 64))
            lo_bc = np.broadcast_to(lo_arr[None, :], (128, 8)).copy()
            xyzq = np.ascontiguousarray(xs[qstart:qstart + 1024].T)
            m = {"feat": feat, "rhs": rhs, "qneg": qneg, "lo": lo_bc,
                 "spq": spq, "xyzq": xyzq, "eye": eye, "rep16": rep16}
            m.update(wmaps)
            in_maps.append(m)

    res = run_bass_kernel_spmd(nc, in_maps, list(range(NCORES)),
                               trace=_TRACE)
    _LAST_EXEC_NS = res.exec_time_ns

    out = np.empty((B, 128, N), f32)
    for g in range(NCORES):
        b, quarter = g // 4, g % 4
        qstart = 1024 * quarter
        cols = orders[b][qstart:qstart + 1024]
        out[b][:, cols] = np.asarray(res.results[g]["out"])
    return out
